# revision 30
# baseline (speedup 1.0000x reference)
"""Trainium2 Bass kernel for nn_BasicTransformerBlock (dense_transformer).

Sharding: 8 cores = 4 samples x 2 token-halves.  Each core's x input is
rolled so its own 1024 query rows are local rows 0:1024 (key/value sets
are permutation-invariant, so any consistent token order works on the
key side).

Design notes (v6, 1034us -> 748us on HW):
  * K>=256 GEMMs (q/v/o projections, PV, GEGLU FFN) run in fp8e4 with
    DoubleRow perf mode: lhsT [K,2,M], rhs [K,2,N] packs 2 contraction
    subtiles per output column, halving PE passes.  The fp8 dual
    ldweights ISA requires M=128 exactly, so the PV stationary
    [w*v | w] is zero-padded to 128 columns (padding is free: matmul
    cost is per output column).  AB (K=64 per head) stays bf16.
  * L2-distance attention (k == q):
      softmax(sim)_ij = m_ij w_j / sum_j m_ij w_j,
      m = exp(2 s AB - SHIFT), w_j = exp(-s AA_j)
    The q-projection bias cancels exactly (sim is shift-invariant) and
    is dropped.  SHIFT keeps m inside fp8e4 range; it cancels in the
    softmax ratio, as does the x64 weight scale folded into [w*v | w].
    The padded PV stationary also yields the denominators (w column);
    each head pair is normalized immediately after its PV so the
    reciprocal/broadcast round-trip overlaps the next pair's matmuls.
  * No DMA transposes: LN outputs are transposed by the PE (identity
    matmul) and copied out of PSUM by the DVE (with fp8 cast).
  * Stage-2 keys: each core AllGathers its own TRANSPOSED fp8 LN2
    block [512 dims x 512 tok]; the gather concatenates the pair's
    blocks, so reading back both positions gives all 2048 keys in
    ORIGINAL token order with no re-transposes and no rank-dependent
    indexing (also fixes a baseline bug where half=1 cores attended to
    their own rows twice in stage 2).  Stage-2 LN + query projections
    run interleaved inside stage-1 attention (post_block); stage-3 LN
    runs inside stage-2 attention.
  * Remaining wall: the PE power throttle (~55% avg util limit) holds
    matmul streaming near 1ns per output column; ACT exp (~133us per
    attention stage) is the next floor.
"""

import sys

sys.path.insert(0, "/opt/trn_rl_repo")

import contextlib
import math

import numpy as np
import ml_dtypes

import concourse.bass as bass
import concourse.tile as tile
import concourse.mybir as mybir
from concourse.bass import ds, ts
from concourse.masks import make_identity
from concourse.vector_clock import ScopedClock

AF = mybir.ActivationFunctionType
ALU = mybir.AluOpType
DR = mybir.MatmulPerfMode.DoubleRow
F32 = mybir.dt.float32
BF16 = mybir.dt.bfloat16
F8 = mybir.dt.float8e4

P = 128
NT = 2048        # tokens per sample
NO = 1024        # tokens owned per core
DIM = 512
NC = DIM // P    # 4 contraction chunks
H = 8
D = 64
FF = 2048        # GEGLU inner dim
EPS = 1e-5
SCALE = D ** -0.5      # 0.125
IB = 512         # attention i-block (query columns per psum round)
SHIFT = 3.0      # exp shift, cancels in softmax ratio; keeps pp in fp8 range
WS = 64.0        # weight fp8 scale
QS = 8.0         # qT8 fp8 scale
LN64 = math.log(WS)

CH_ROWS = 512
NCH = NO // CH_ROWS    # 2 gather chunks


def _patch_tile_drain():
    """This walrus build rejects sem waits on the SP Drain that TileContext
    emits at kernel end ("Too many sync wait commands").  Put every wait on
    its own preceding sync-engine nop instead."""
    if getattr(tile.TileContext, "_drain_patched", False):
        return

    def _drain_and_barrier(self, tick_clock, wait_clock):
        nc = self.nc
        carriers = [nc.sync.nop(hint=f"dw{i}", nofuse=True) for i in range(28)]
        drain_inst = nc.sync.drain()
        wait_clock.add_sem_waits(
            drain_inst.ins, ScopedClock({None: tick_clock.global_clock})
        )
        si = drain_inst.ins.sync_info
        waits = list(si.on_wait) if si is not None else []
        drain_inst.ins.sync_info = mybir.SyncInfo(
            on_wait=[], on_update=list(si.on_update) if si else []
        )
        assert len(waits) <= len(carriers)
        for i, w in enumerate(waits):
            carriers[i].ins.sync_info = mybir.SyncInfo(on_wait=[w], on_update=[])
        nc.all_engine_barrier()
        assert self.sems is not None
        popped = nc._tile_sem_poison_stack.pop()
        assert popped is self._sem_poison
        nc.clear_and_free_semaphores(list(self.sems.allocated().values()))
        nc.all_engine_barrier()

    tile.TileContext._drain_and_barrier = _drain_and_barrier
    tile.TileContext._drain_patched = True


_WAIT_CAPS = {
    "InstDrain": 0,
}
_WAIT_CAP_DEFAULT = 1


def _split_excess_waits(nc):
    """Hoist sem waits beyond an instruction's capacity onto same-engine NoOps
    inserted immediately before it."""
    uid = [0]
    for f in nc.m.functions:
        for bb in f.blocks:
            insts = list(bb.instructions)
            out = []
            for inst in insts:
                si = inst.sync_info
                waits = list(si.on_wait) if si is not None else []
                cap = _WAIT_CAPS.get(type(inst).__name__, _WAIT_CAP_DEFAULT)
                if len(waits) > cap:
                    keep = waits[:cap]
                    rest = waits[cap:]
                    for w in rest:
                        uid[0] += 1
                        out.append(mybir.InstNoOp(
                            name=f"WSPL-{uid[0]}",
                            engine=inst.engine,
                            bass_nofuse=True,
                            sync_info=mybir.SyncInfo(on_wait=[w], on_update=[]),
                        ))
                    inst.sync_info = mybir.SyncInfo(
                        on_wait=keep,
                        on_update=list(si.on_update) if si else [])
                out.append(inst)
            if len(out) != len(insts):
                bb.instructions = out


def _bcast_ap(dram_ap, p=P):
    """Partition-broadcast read AP for a DMA: replicate a DRAM vector across
    p partitions."""
    if not isinstance(dram_ap, bass.AP):
        dram_ap = dram_ap[:]
    return bass.AP(tensor=dram_ap.tensor, offset=dram_ap.offset,
                   ap=[[0, p]] + [list(x) for x in dram_ap.ap])


def build_program():
    _patch_tile_drain()
    nc = bass.Bass(trn_type="TRN2", target_bir_lowering=False, num_swdge_queues=4)

    x_dram = nc.dram_tensor("x", [NT, DIM], F32, kind="ExternalInput")
    wints = {}
    for st in (1, 2):
        for nm in ("wq", "wv", "wo"):
            wints[f"{nm}{st}"] = nc.dram_tensor(f"{nm}{st}", [DIM, DIM], F8,
                                                kind="ExternalInput")
        for nm in ("bv", "bo"):
            wints[f"{nm}{st}"] = nc.dram_tensor(f"{nm}{st}", [DIM], F32,
                                                kind="ExternalInput")
    w1_dram = nc.dram_tensor("w1", [DIM, 2 * FF], F8, kind="ExternalInput")
    b1_dram = nc.dram_tensor("b1", [2 * FF], F32, kind="ExternalInput")
    w2_dram = nc.dram_tensor("w2", [FF, DIM], F8, kind="ExternalInput")
    b2_dram = nc.dram_tensor("b2", [DIM], F32, kind="ExternalInput")
    out_dram = nc.dram_tensor("out", [NO, DIM], F32, kind="ExternalOutput")

    with tile.TileContext(nc) as tc:
        _emit(nc, tc, x_dram, wints, w1_dram, b1_dram, w2_dram, b2_dram, out_dram)
    _split_excess_waits(nc)
    return nc


def _emit(nc, tc, x_dram, wints, w1_dram, b1_dram, w2_dram, b2_dram, out_dram):
    ctx = contextlib.ExitStack()
    with ctx:
        state = ctx.enter_context(tc.tile_pool(name="state", bufs=1))
        tmp = ctx.enter_context(tc.tile_pool(name="tmp", bufs=2))
        lnp = ctx.enter_context(tc.tile_pool(name="lnp", bufs=6))
        small = ctx.enter_context(tc.tile_pool(name="small", bufs=4))
        ppool = ctx.enter_context(tc.tile_pool(name="pp", bufs=6))
        rbp = ctx.enter_context(tc.tile_pool(name="rbp", bufs=4))
        dram = ctx.enter_context(tc.tile_pool(name="dram", bufs=2, space="DRAM"))
        psum_ab = ctx.enter_context(tc.tile_pool(name="ps_ab", bufs=2, space="PSUM"))
        psum_pv = ctx.enter_context(tc.tile_pool(name="ps_pv", bufs=1, space="PSUM"))
        psum_tr = ctx.enter_context(tc.tile_pool(name="ps_tr", bufs=2, space="PSUM"))

        # ---------------- whole-kernel state -----------------
        trunk = state.tile([P, NO // P, DIM], F32, tag="trunk")
        # x rows first: the LN stats chain depends on them
        nc.gpsimd.dma_start(
            trunk, x_dram[0:NO].rearrange("(n p) d -> p n d", p=P))
        xpart = state.tile([P, 8, DIM], F32, tag="xpart")      # partner x rows
        nc.sync.dma_start(
            xpart[:, 0:4], x_dram[NO:NO + 512].rearrange("(n p) d -> p n d", p=P))
        nc.sync.dma_start(
            xpart[:, 4:8], x_dram[NO + 512:NT].rearrange("(n p) d -> p n d", p=P))
        xnT8 = state.tile([P, NC, NT], F8, tag="xnT8")         # keys LN^T (fp8)
        xnTq8 = state.tile([P, NC, NO], F8, tag="xnTq8")       # query-side LN^T
        qTb = state.tile([P, NC, NT], BF16, tag="qTb")         # keys q^T
        qTq2b = state.tile([P, NC, NO], BF16, tag="qTq2b")     # stage2 query q^T
        # PV stationary, M=128 padded (fp8 DoubleRow requires 128 cols):
        #  even heads: [v*w (0:64) | w (64) | zeros];
        #  odd heads:  [zeros | w (32) | zeros | v*w (64:128)]
        vext8 = state.tile([P, 8, 2, H, P], F8, tag="vext8")
        nc.gpsimd.memset(vext8, 0.0)
        ww = state.tile([P, NT // P, H], F32, tag="ww")        # w_j per head (x64)
        otn8 = state.tile([P, 4, NO], F8, tag="otn8")
        ueall = state.tile([P, 4, IB], BF16, tag="ueall")
        dn = state.tile([P, IB], F32, tag="dn")
        ident = state.tile([P, P], BF16, tag="ident")
        make_identity(nc, ident)
        eps_t = state.tile([P, 1], F32, tag="eps")
        nc.vector.memset(eps_t, EPS)
        nsh_t = state.tile([P, 1], F32, tag="nsh")
        nc.vector.memset(nsh_t, -SHIFT)
        ln64_t = state.tile([P, 1], F32, tag="ln64")
        nc.vector.memset(ln64_t, LN64)

        # ---- weights, all prefetched up front ----
        wq8t, wv8t, wo8t, bvb, bob = {}, {}, {}, {}, {}
        for st in (1, 2):
            for nm, store in (("wq", wq8t), ("wv", wv8t), ("wo", wo8t)):
                t = state.tile([P, NC, DIM], F8, tag=f"{nm}{st}",
                               name=f"{nm}{st}t")
                eng = nc.gpsimd if st == 1 else nc.sync
                eng.dma_start(
                    t, wints[f"{nm}{st}"].rearrange("(c p) o -> p c o", p=P))
                store[st] = t
            bvb[st] = state.tile([P, DIM], F32, tag=f"bvb{st}",
                                 name=f"bvb{st}")
            nc.gpsimd.dma_start(bvb[st], _bcast_ap(wints[f"bv{st}"]))
            bob[st] = state.tile([P, DIM], F32, tag=f"bob{st}",
                                 name=f"bob{st}")
            nc.gpsimd.dma_start(bob[st], _bcast_ap(wints[f"bo{st}"]))
        w1t8 = state.tile([P, NC, 2 * FF], F8, tag="w1t8")
        nc.sync.dma_start(w1t8, w1_dram.rearrange("(c p) o -> p c o", p=P))
        w2t8 = state.tile([P, FF // P, DIM], F8, tag="w2t8")
        nc.sync.dma_start(w2t8, w2_dram.rearrange("(c p) o -> p c o", p=P))
        b1t = state.tile([P, 2 * FF // P], F32, tag="b1t")
        nc.sync.dma_start(b1t, b1_dram.rearrange("(c p) -> p c", p=P))
        b2b = state.tile([P, DIM], F32, tag="b2b")
        nc.sync.dma_start(b2b, _bcast_ap(b2_dram))
        mT8 = state.tile([P, FF // P, NO], F8, tag="mT8")


        # ---------------- helpers -----------------
        def ln_stats(srcs, tag="g4"):
            """Stats + one batched sqrt for the whole group -> (mvs, si)."""
            g = len(srcs)
            mvs = small.tile([P, g, 2], F32, tag="mvs" + tag)
            for i, src in enumerate(srcs):
                st = small.tile([P, nc.vector.BN_STATS_DIM], F32, tag="bnst")
                nc.vector.bn_stats(st, src)
                nc.vector.bn_aggr(mvs[:, i], st)
            sd = small.tile([P, g], F32, tag="sd" + tag)
            nc.scalar.activation(sd, mvs[:, :, 1], AF.Sqrt, bias=eps_t)
            si = small.tile([P, g], F32, tag="si" + tag)
            nc.vector.reciprocal(si, sd)
            return mvs, si

        def ln_apply(src, mvs, si, i):
            """(src - mean) * istd -> fresh bf16 tile (affine folded into
            the projection weights host-side)."""
            xn = lnp.tile([P, DIM], BF16, tag="xn")
            nc.vector.tensor_scalar(xn, src, mvs[:, i, 0:1], si[:, i:i + 1],
                                    op0=ALU.subtract, op1=ALU.mult)
            return xn

        def transpose_cast(dst8, xn):
            """xn [128 tok, 512 dim] bf16 -> dst8 = xnT8[:, :, tok-tile] fp8."""
            pt = psum_tr.tile([P, DIM], BF16, tag="tr")
            for c in range(NC):
                nc.tensor.transpose(pt[:, ts(c, P)], xn[:, ts(c, P)], ident)
            nc.vector.tensor_copy(
                dst8, pt.rearrange("p (c t) -> p c t", c=NC))

        def qt_proj_tsl(xsrc8, tsl, wq8, dstq):
            """Project one 512-token slice into qT chunks (bf16)."""
            for cg in range(2):
                pq = psum_ab.tile([P, 1024], F32, tag="ab")
                for sub in range(2):
                    t = 2 * cg + sub
                    for kc in (0, 2):
                        nc.tensor.matmul(
                            pq[:, sub * 512:(sub + 1) * 512],
                            lhsT=wq8[:, kc:kc + 2, ds(t * P, P)],
                            rhs=xsrc8[:, kc:kc + 2, ds(tsl * 512, 512)],
                            perf_mode=DR, start=(kc == 0), stop=(kc == 2))
                nc.vector.tensor_scalar_mul(
                    dstq[:, 2 * cg:2 * cg + 2, ds(tsl * 512, 512)],
                    pq.rearrange("p (s n) -> p s n", s=2), 1.0 / WS)

        def v_and_w(xsrc8, n, wq8, wv8, bvb_t):
            """Per key tile: v (+bias), AA -> ww, fold [w*v | w] into vext8."""
            pq = psum_ab.tile([P, 1024], F32, tag="ab")
            for kc in (0, 2):
                nc.tensor.matmul(
                    pq[:, 0:512],
                    lhsT=xsrc8[:, kc:kc + 2, ts(n, P)],
                    rhs=wq8[:, kc:kc + 2, :],
                    perf_mode=DR, start=(kc == 0), stop=(kc == 2))
            for kc in (0, 2):
                nc.tensor.matmul(
                    pq[:, 512:1024],
                    lhsT=xsrc8[:, kc:kc + 2, ts(n, P)],
                    rhs=wv8[:, kc:kc + 2, :],
                    perf_mode=DR, start=(kc == 0), stop=(kc == 2))
            vtile = tmp.tile([P, DIM], F32, tag="vtile")
            nc.vector.scalar_tensor_tensor(
                vtile, pq[:, 512:1024], 1.0 / WS, bvb_t,
                op0=ALU.mult, op1=ALU.add)
            sq = tmp.tile([P, DIM], F32, tag="sq")
            nc.scalar.activation(sq, pq[:, 0:512], AF.Square)
            aa = small.tile([P, H], F32, tag="aa")
            nc.vector.tensor_reduce(
                aa, sq.rearrange("p (h d) -> p h d", d=D),
                axis=mybir.AxisListType.X, op=ALU.add)
            # ww = 64 * exp(-s*AA);  psum q carries x64 -> AA x4096
            nc.scalar.activation(ww[:, n, :], aa, AF.Exp,
                                 scale=-SCALE / (WS * WS), bias=ln64_t)
            ve = vext8[:, n // 2, n % 2]
            wwn = ww[:, n, :].rearrange("p (h u) -> p h u", u=1)
            vv = vtile.rearrange("p (h d) -> p h d", d=D)
            # even heads: v*w at cols 0:64, w at col 64
            nc.vector.tensor_tensor(
                ve[:, 0:H:2, 0:64], vv[:, 0:H:2],
                wwn[:, 0:H:2].to_broadcast([P, H // 2, D]), op=ALU.mult)
            nc.vector.tensor_copy(ve[:, 0:H:2, 64:65], wwn[:, 0:H:2])
            # odd heads: v*w at cols 64:128, w at col 32
            nc.vector.tensor_tensor(
                ve[:, 1:H:2, 64:128], vv[:, 1:H:2],
                wwn[:, 1:H:2].to_broadcast([P, H // 2, D]), op=ALU.mult)
            nc.vector.tensor_copy(ve[:, 1:H:2, 32:33], wwn[:, 1:H:2])

        # Transposed-fp8 gather: each core ships its own xnT tile block
        # [512 dims, 512 tok]; AllGather stacks the pair's blocks, which
        # read back as the full 2048 keys in ORIGINAL token order
        # (rank-independent).
        g_ins, g_outs = {}, {}
        for stg in (2,):
            g_ins[stg] = dram.tile([NCH, DIM, CH_ROWS], F8, tag=f"g{stg}in",
                                   name=f"g{stg}in")
            g_outs[stg] = [dram.tile([2 * DIM, CH_ROWS], F8, tag=f"g{stg}o{ch}",
                                     name=f"g{stg}o{ch}") for ch in range(NCH)]

        def launch_gatherT(stg, ch):
            nc.gpsimd.dma_start(
                g_ins[stg][ch].rearrange("(c p) t -> p c t", p=P),
                xnTq8[:, :, ds(ch * CH_ROWS, CH_ROWS)])
            nc.gpsimd.collective_compute(
                kind="AllGather", op=ALU.bypass,
                replica_groups=[[0, 1], [2, 3], [4, 5], [6, 7]],
                ins=[g_ins[stg][ch]],
                outs=[g_outs[stg][ch][:]])

        def keys_build(stg, wq8, wv8, bvb_t):
            """Read back both gather positions into xnT8 (original order)
            and project keys q/v per chunk."""
            for ch in range(NCH):
                go = g_outs[stg][ch]
                nc.gpsimd.dma_start(
                    xnT8[:, :, ds(ch * CH_ROWS, CH_ROWS)],
                    go[0:DIM].rearrange("(c p) t -> p c t", p=P))
                nc.gpsimd.dma_start(
                    xnT8[:, :, ds(NO + ch * CH_ROWS, CH_ROWS)],
                    go[DIM:2 * DIM].rearrange("(c p) t -> p c t", p=P))
                for tsl in (ch, 2 + ch):
                    qt_proj_tsl(xnT8, tsl, wq8, qTb)
                    for n in range(4 * tsl, 4 * tsl + 4):
                        v_and_w(xnT8, n, wq8, wv8, bvb_t)

        def pb_stage2(ch):
            """Inside attention-1: LN2 own chunk, ship transposed to the
            pair, project stage-2 queries."""
            srcs = [trunk[:, 4 * ch + i] for i in range(4)]
            mvs, si = ln_stats(srcs)
            for i, src in enumerate(srcs):
                xn = ln_apply(src, mvs, si, i)
                transpose_cast(xnTq8[:, :, ts(4 * ch + i, P)], xn)
            launch_gatherT(2, ch)
            qt_proj_tsl(xnTq8, ch, wq8t[2], qTq2b)

        def pb_stage3(ch):
            """Inside attention-2: LN3 own chunk -> xnT8 for the FFN."""
            srcs = [trunk[:, 4 * ch + i] for i in range(4)]
            mvs, si = ln_stats(srcs)
            for i, src in enumerate(srcs):
                xn = ln_apply(src, mvs, si, i)
                transpose_cast(xnT8[:, :, ts(4 * ch + i, P)], xn)

        # =================== attention =====================
        def attention(keys8, qsrc8, wo8, bob_t, post_block):
            n_jp = NT // (2 * P)   # 8 jpairs
            for ib0 in range(NO // IB):
                for hp in range(4):
                    h_e, h_o = 2 * hp, 2 * hp + 1
                    ppv = psum_pv.tile([P, 1024], F32, tag="pv")

                    def emit_pv(jp, pp_e, pp_o):
                        st, sp = (jp == 0), (jp == n_jp - 1)
                        nc.tensor.matmul(
                            ppv[:, 0:512],
                            lhsT=vext8[:, jp, :, h_e, :],
                            rhs=pp_e.rearrange("p (s n) -> p s n", s=2),
                            perf_mode=DR, start=st, stop=sp)
                        nc.tensor.matmul(
                            ppv[:, 512:1024],
                            lhsT=vext8[:, jp, :, h_o, :],
                            rhs=pp_o.rearrange("p (s n) -> p s n", s=2),
                            perf_mode=DR, start=st, stop=sp)

                    pending = None
                    for jp in range(n_jp):
                        pab_e = psum_ab.tile([P, 1024], F32, tag="ab")
                        pab_o = psum_ab.tile([P, 1024], F32, tag="ab")
                        for s in range(2):
                            j = 2 * jp + s
                            nc.tensor.matmul(
                                pab_e[:, s * 512:(s + 1) * 512],
                                lhsT=keys8[0:64, hp, ts(j, P)],
                                rhs=qsrc8[0:64, hp, ds(ib0 * IB, IB)],
                                start=True, stop=True)
                            nc.tensor.matmul(
                                pab_o[:, s * 512:(s + 1) * 512],
                                lhsT=keys8[64:128, hp, ts(j, P)],
                                rhs=qsrc8[64:128, hp, ds(ib0 * IB, IB)],
                                start=True, stop=True)
                        if pending is not None:
                            emit_pv(*pending)
                        pp_e = ppool.tile([P, 1024], F8, tag="pp")
                        nc.scalar.activation(pp_e, pab_e, AF.Exp,
                                             scale=2.0 * SCALE,
                                             bias=nsh_t)
                        pp_o = ppool.tile([P, 1024], F8, tag="pp")
                        nc.scalar.activation(pp_o, pab_o, AF.Exp,
                                             scale=2.0 * SCALE,
                                             bias=nsh_t)
                        pending = (jp, pp_e, pp_o)
                    emit_pv(*pending)
                    # unnormalized out + denominators -> sbuf; normalize
                    # this head pair immediately (overlaps the next pair's
                    # AB/PV instead of a global round-trip at ib0 end)
                    nc.vector.tensor_copy(ueall[0:64, hp, :], ppv[0:64, 0:512])
                    nc.vector.tensor_copy(ueall[64:128, hp, :],
                                          ppv[64:128, 512:1024])
                    dsb = tmp.tile([P, IB], F32, tag="dsb")
                    nc.vector.tensor_copy(dsb[64:65, :], ppv[64:65, 0:512])
                    nc.vector.tensor_copy(dsb[32:33, :], ppv[32:33, 512:1024])
                    # collect the pair's dens on adjacent 32-aligned rows so
                    # one reciprocal covers both (recip is ~3us/instr)
                    b32 = 32 * hp
                    nc.gpsimd.dma_start(dn[b32:b32 + 1, :], dsb[64:65, :])
                    nc.gpsimd.dma_start(dn[b32 + 1:b32 + 2, :], dsb[32:33, :])
                    nc.vector.reciprocal(dn[b32:b32 + 2, :], dn[b32:b32 + 2, :])
                    dnd = dram.tile([2, IB], F32, tag="dnd")
                    nc.gpsimd.dma_start(dnd, dn[b32:b32 + 2, :])
                    for hl in range(2):
                        rb = rbp.tile([P, IB], F32, tag="rb")
                        nc.gpsimd.dma_start(rb, _bcast_ap(dnd[hl]))
                        b0 = 64 * hl
                        nc.vector.tensor_tensor(
                            otn8[b0:b0 + 64, hp, ds(ib0 * IB, IB)],
                            ueall[b0:b0 + 64, hp, :], rb[b0:b0 + 64, :],
                            op=ALU.mult)
                # O-projection + bias + residual
                for isl in range(ib0 * (IB // P), (ib0 + 1) * (IB // P)):
                    po = psum_ab.tile([P, 1024], F32, tag="ab")
                    for sp2 in (0, 2):
                        nc.tensor.matmul(
                            po[:, 0:512],
                            lhsT=otn8[:, sp2:sp2 + 2, ts(isl, P)],
                            rhs=wo8[:, sp2:sp2 + 2, :],
                            perf_mode=DR, start=(sp2 == 0), stop=(sp2 == 2))
                    xr = tmp.tile([P, DIM], F32, tag="xr")
                    nc.vector.scalar_tensor_tensor(
                        xr, po[:, 0:512], 1.0 / WS, bob_t,
                        op0=ALU.mult, op1=ALU.add)
                    nc.vector.tensor_tensor(trunk[:, isl], xr, trunk[:, isl],
                                            op=ALU.add)
                    if post_block is not None and isl % 4 == 3:
                        post_block(isl // 4)

        # ---- stage-1 setup: LN all 2048 rolled rows locally ----
        srcs1 = [trunk[:, i] for i in range(8)] + [xpart[:, i] for i in range(8)]
        mvs1, si1 = ln_stats(srcs1, tag="g16")
        for n, src in enumerate(srcs1):
            xn = ln_apply(src, mvs1, si1, n)
            transpose_cast(xnT8[:, :, ts(n, P)], xn)
            if n % 4 == 3:
                qt_proj_tsl(xnT8, n // 4, wq8t[1], qTb)
                for n2 in range(n - 3, n + 1):
                    v_and_w(xnT8, n2, wq8t[1], wv8t[1], bvb[1])

        # ---- stage 1 attention (keys rolled; queries = own half) ----
        attention(qTb, qTb, wo8t[1], bob[1], pb_stage2)

        # ---- stage-2 keys from the LN2 gather ----
        keys_build(2, wq8t[2], wv8t[2], bvb[2])

        # ---- stage 2 attention (keys original order; queries own) ----
        attention(qTb, qTq2b, wo8t[2], bob[2], pb_stage3)

        # ================= stage 3 (GEGLU FFN, own rows) =================
        NS = FF // P  # 16 inner slices
        for s in range(NS):
            pa = psum_ab.tile([P, 1024], F32, tag="ab")
            pg = psum_pv.tile([P, 1024], F32, tag="pv")
            for tsl in range(NO // 512):
                for kc in (0, 2):
                    nc.tensor.matmul(
                        pa[:, tsl * 512:(tsl + 1) * 512],
                        lhsT=w1t8[:, kc:kc + 2, ds(s * P, P)],
                        rhs=xnT8[:, kc:kc + 2, ds(tsl * 512, 512)],
                        perf_mode=DR, start=(kc == 0), stop=(kc == 2))
                for kc in (0, 2):
                    nc.tensor.matmul(
                        pg[:, tsl * 512:(tsl + 1) * 512],
                        lhsT=w1t8[:, kc:kc + 2, ds(FF + s * P, P)],
                        rhs=xnT8[:, kc:kc + 2, ds(tsl * 512, 512)],
                        perf_mode=DR, start=(kc == 0), stop=(kc == 2))
            gt = tmp.tile([P, NO], BF16, tag="gt")
            nc.scalar.activation(gt, pg[:, 0:NO], AF.Gelu,
                                 scale=1.0 / WS, bias=b1t[:, NS + s:NS + s + 1])
            at_ = tmp.tile([P, NO], BF16, tag="at")
            nc.scalar.activation(at_, pa[:, 0:NO], AF.Identity,
                                 scale=1.0 / WS, bias=b1t[:, s:s + 1])
            nc.vector.tensor_tensor(mT8[:, s, :], at_, gt, op=ALU.mult)
        for tsl in range(NO // P):
            py = psum_ab.tile([P, 1024], F32, tag="ab")
            for mk in range(0, NS, 2):
                nc.tensor.matmul(
                    py[:, 0:512],
                    lhsT=mT8[:, mk:mk + 2, ts(tsl, P)],
                    rhs=w2t8[:, mk:mk + 2, :],
                    perf_mode=DR, start=(mk == 0), stop=(mk == NS - 2))
            yt = tmp.tile([P, DIM], F32, tag="yt")
            nc.vector.scalar_tensor_tensor(
                yt, py[:, 0:512], 1.0 / WS, b2b, op0=ALU.mult, op1=ALU.add)
            nc.vector.tensor_tensor(yt, yt, trunk[:, tsl], op=ALU.add)
            nc.gpsimd.dma_start(
                out_dram.rearrange("(n p) d -> p n d", p=P)[:, tsl], yt)


_CACHE = {}


def _get_program():
    if "nc" not in _CACHE:
        _CACHE["nc"] = build_program()
    return _CACHE["nc"]


# wq output-column permutation for the DoubleRow qT layout:
# new col t*128 + b*32 + k  <-  head (t//2)*4 + b, dim (t%2)*32 + k
def _wq_perm():
    perm = np.empty(DIM, np.int64)
    for t in range(4):
        for b in range(4):
            for k in range(32):
                perm[t * 128 + b * 32 + k] = ((t // 2) * 4 + b) * 64 \
                    + (t % 2) * 32 + k
    return perm


_PERM = _wq_perm()


def _prep_inputs(inputs):
    """Host-side: fold LN affine params into projection weights, quantize
    weights to fp8e4 (x64), permute wq columns for the DR qT layout.
    The q bias is dropped: the L2 attention similarity is invariant to a
    common shift of q."""
    f = np.float32
    f8 = ml_dtypes.float8_e4m3fn

    def fold_w(nw, w):
        return (np.asarray(nw, f)[:, None] * np.asarray(w, f))

    def to8(w):
        return (np.asarray(w, f) * WS).astype(f8)

    base = {}
    for st, pre in ((1, "a1"), (2, "a2")):
        nrm = ("sa" if st == 1 else "ca")
        nw, nb = inputs[f"{nrm}_norm_w"], inputs[f"{nrm}_norm_b"]
        wq = fold_w(nw, inputs[f"{pre}_wq"])
        base[f"wq{st}"] = to8(wq)
        wv = fold_w(nw, inputs[f"{pre}_wv"])
        base[f"wv{st}"] = to8(wv)
        base[f"bv{st}"] = (np.asarray(nb, f) @ np.asarray(inputs[f"{pre}_wv"], f)
                           + np.asarray(inputs[f"{pre}_bv"], f)).astype(f)
        base[f"wo{st}"] = to8(inputs[f"{pre}_wo"])
        base[f"bo{st}"] = np.asarray(inputs[f"{pre}_bo"], f)
    nw, nb = inputs["ff_norm_w"], inputs["ff_norm_b"]
    base["w1"] = to8(fold_w(nw, inputs["ff_w1"]))
    base["b1"] = (np.asarray(nb, f) @ np.asarray(inputs["ff_w1"], f)
                  + np.asarray(inputs["ff_b1"], f)).astype(f)
    base["w2"] = to8(inputs["ff_w2"])
    base["b2"] = np.asarray(inputs["ff_b2"], f)
    return base


def _run(inputs, trace=False, trace_kwargs=None):
    from concourse.bass_utils import run_bass_kernel_spmd

    nc = _get_program()
    x = np.ascontiguousarray(inputs["x"], dtype=np.float32)
    base = _prep_inputs(inputs)
    in_maps = []
    for c in range(8):
        b, half = c // 2, c % 2
        m = dict(base)
        m["x"] = np.ascontiguousarray(np.roll(x[b], -NO * half, axis=0))
        in_maps.append(m)
    kw = {}
    if trace:
        kw = dict(trace=True, trace_kwargs=trace_kwargs or {})
    res = run_bass_kernel_spmd(nc, in_maps, core_ids=list(range(8)), **kw)
    out = np.empty((4, NT, DIM), np.float32)
    for c in range(8):
        b, half = c // 2, c % 2
        out[b, half * NO:(half + 1) * NO] = res.results[c]["out"]
    return out, res


def kernel(**inputs):
    out, _ = _run(inputs)
    return out


if __name__ == "__main__":
    import reference
    ins = {k: np.asarray(v) for k, v in reference.setup_inputs().items()}
    out = kernel(**ins)
    exp = np.asarray(reference.reference(**ins))
    err = np.abs(out - exp).max() / (np.abs(exp).max() + 1e-9)
    print("Relative error:", err)
